# revision 6
# baseline (speedup 1.0000x reference)
"""Trainium2 Bass kernel for nn_MemristiveLinear.

The reference's differential-conductance-pair math collapses exactly:
  g_pos - g_neg = k_cond * weights   (the G_OFF leak terms cancel)
so total_currents = K_V * inputs @ (k_cond * weights) and
  y = total_currents / (K_V * k_cond) = inputs @ weights = x @ w + b.

Device kernel: y = x @ w + b, sharded over 8 NeuronCores in a
2 (batch) x 4 (n_out) grid.  Per core:
  yT_block[128 n_out, 256 batch] = w_shard.T @ x_shardT (+ bias)
with the contraction dim (n_in = 512) split into 4 PSUM-accumulated
128-deep matmuls.

All operands travel as bf16 (the 2e-2 rel-err budget dwarfs bf16's
~4e-3), halving HBM traffic vs fp32 — the kernel is DMA-latency and
DMA-bandwidth bound, not PE bound.  The host packs each core's whole
input into ONE [128, 1538] bf16 DRAM tensor, contiguous per SBUF
partition:
  per partition p: [w0 128 | x0 256 | w1 | x1 | w2 | x2 | w3 | x3 | b 1 | pad 1]
where w_ko[p, m] = w[ko*128+p, m] and x_ko[p, n] = x[n, ko*128+p].
The input moves as 2 DMAs (split at the ko=2 boundary) so the first
two matmuls overlap the second half's transfer.

The output is split into two 128-batch-column halves, each with its
own PSUM accumulation chain, so half A's PSUM->SBUF bias-add and
store DMA overlap half B's matmuls; the stores are issued from the
Activation engine's HWDGE (nc.scalar) so their descriptor generation
doesn't queue behind the Sync engine's input DMAs.
"""

import numpy as np
import ml_dtypes

import concourse.bacc as bacc
import concourse.mybir as mybir
import concourse.tile as tile
from concourse.bass_utils import run_bass_kernel_spmd

N_CORES = 8
B, NIN, NOUT = 512, 512, 512
GB, GN = 2, 4            # batch groups x n_out groups
BS, NS = B // GB, NOUT // GN   # 256 batch rows, 128 n_out cols per core
P = 128
KO = NIN // P            # 4 contraction blocks
CHUNK = NS + BS          # 384 bf16 per ko chunk (w block + x block)
INW = KO * CHUNK + 2     # 1538 bf16 per partition (bias + pad)
HB = BS // 2             # 128-batch-column output halves

_NC = None


def _build():
    nc = bacc.Bacc("TRN2", target_bir_lowering=False, debug=False,
                   num_devices=N_CORES)
    f32 = mybir.dt.float32
    bf16 = mybir.dt.bfloat16
    inp = nc.dram_tensor("inp", [P, INW], bf16, kind="ExternalInput")
    y = nc.dram_tensor("y", [NS, BS], bf16, kind="ExternalOutput")

    with tile.TileContext(nc) as tc:
        with (
            tc.tile_pool(name="sbuf", bufs=1) as pool,
            tc.tile_pool(name="psum", bufs=1, space="PSUM") as psum_pool,
        ):
            in_t = pool.tile([P, INW], bf16, tag="in")
            out_t = pool.tile([NS, BS], bf16, tag="out")
            ps_a = psum_pool.tile([NS, HB], f32, tag="psA")
            ps_b = psum_pool.tile([NS, HB], f32, tag="psB")

            # input: 2 DMAs split at the ko=2 boundary
            mid = 2 * CHUNK
            nc.sync.dma_start(in_t[:, 0:mid], inp.ap()[:, 0:mid])
            nc.sync.dma_start(in_t[:, mid:INW], inp.ap()[:, mid:INW])

            # bias is stored as a float32 in two bf16 slots (4-byte aligned)
            b_t = in_t[:, KO * CHUNK:KO * CHUNK + 2].bitcast(f32)
            for half, ps in ((0, ps_a), (1, ps_b)):
                xlo = NS + half * HB
                for ko in range(KO):
                    base = ko * CHUNK
                    nc.tensor.matmul(ps[:],
                                     in_t[:, base:base + NS],
                                     in_t[:, base + xlo:base + xlo + HB],
                                     start=(ko == 0), stop=(ko == KO - 1))
                o = out_t[:, half * HB:(half + 1) * HB]
                nc.vector.tensor_scalar_add(o, ps[:], b_t)
                nc.scalar.dma_start(y.ap()[:, half * HB:(half + 1) * HB], o)

    nc.compile()
    return nc


def _get_nc():
    global _NC
    if _NC is None:
        _NC = _build()
    return _NC


def _pack_core(xT, w, b, gb, gn):
    """Pack one core's inputs into the [P, INW] bf16 layout."""
    t = np.zeros((P, INW), ml_dtypes.bfloat16)
    xs = xT[:, gb * BS:(gb + 1) * BS]        # [NIN, BS]
    ws = w[:, gn * NS:(gn + 1) * NS]         # [NIN, NS]
    for ko in range(KO):
        base = ko * CHUNK
        rows = slice(ko * P, (ko + 1) * P)
        t[:, base:base + NS] = ws[rows]
        t[:, base + NS:base + CHUNK] = xs[rows]
    # bias: raw float32 bytes across the last two bf16 slots
    bia = np.ascontiguousarray(b[gn * NS:(gn + 1) * NS], dtype=np.float32)
    t.view(np.uint16)[:, KO * CHUNK:KO * CHUNK + 2] = (
        bia.view(np.uint32)[:, None] >> np.array([0, 16], np.uint32)[None, :]
    ).astype(np.uint16)
    return t


def _make_in_maps(x, w, b):
    xT = np.ascontiguousarray(np.asarray(x, dtype=np.float32).T).astype(
        ml_dtypes.bfloat16)
    w = np.asarray(w, dtype=np.float32).astype(ml_dtypes.bfloat16)
    b = np.asarray(b, dtype=np.float32)
    in_maps = []
    for c in range(N_CORES):
        gb, gn = divmod(c, GN)
        in_maps.append({"inp": _pack_core(xT, w, b, gb, gn)})
    return in_maps


def _gather(results):
    y = np.empty((B, NOUT), np.float32)
    for c in range(N_CORES):
        gb, gn = divmod(c, GN)
        blk = results[c]["y"].astype(np.float32)
        y[gb * BS:(gb + 1) * BS, gn * NS:(gn + 1) * NS] = blk.T
    return y


def run(x, w, b, **spmd_kwargs):
    """Run on hardware; returns (y, BassKernelResults)."""
    nc = _get_nc()
    res = run_bass_kernel_spmd(nc, _make_in_maps(x, w, b),
                               list(range(N_CORES)), **spmd_kwargs)
    return _gather(res.results), res


def kernel(x, w, b):
    y, _ = run(x, w, b)
    return y
